# revision 1
# baseline (speedup 1.0000x reference)
"""Trainium2 Bass kernel: DifferentiableKendallTau loss.

Reference computes tau = mean over strict-upper-triangle of
tanh((p_j - p_i) * (t_j - t_i) / T) for the flattened n=8192 inputs.

Device strategy (8 NeuronCores, SPMD — one program, per-core data):
  * M[i,j] = (p_j-p_i)(t_j-t_i) is rank-4:  M = 1*u^T + u*1^T - p*t^T - t*p^T
    with u = p*t.  Each fp32 factor is split hi+lo into bf16 (exact products,
    fp32 PSUM accumulation) -> a rank-16 bf16 matmul reproduces M to ~1e-7.
  * TensorE builds 128x512 blocks of M into PSUM (K=16 matmuls).
  * ScalarE computes tanh(10*x) over each PSUM block with accum_out
    reduction into a per-call stats column.
  * Triangle: each core covers 8 of the 64 row-blocks (balanced pairing
    bi=k / bi=63-k), columns strictly right of the diagonal block
    (zero-padded to 512-col chunks; tanh(0)=0 so padding is free), plus its
    8 diagonal blocks in a separate half-weighted pass.
  * Host sums the tiny per-core stats and divides by the pair count.
"""

import numpy as np
import ml_dtypes

import concourse.bass as bass
import concourse.bacc as bacc
import concourse.tile as tile
from concourse import mybir
from concourse.bass_utils import run_bass_kernel_spmd

N = 8192
NB = N // 128            # 64 row-blocks
NCORES = 8
TEMP_INV = 10.0          # 1 / TEMPERATURE
K = 16                   # rank after bf16 hi/lo split of 4 fp32 factors
NJOBS = 66               # 512-col off-diag jobs per core (identical on all cores)
NGROUPS = 17             # 16 groups of 4 jobs + 1 group of 2 (ACT granularity)
NDIAG = 8                # diagonal 128x128 blocks per core
NSTAT = NGROUPS + 1

_CACHE = {}


def _core_blocks(c):
    ks = [4 * c + r for r in range(4)]
    return ks + [63 - k for k in ks]


def _jobs_for_core(c):
    """(row_block, col_start, width<=512) jobs covering columns strictly right
    of each row-block's diagonal block. 66 jobs for every core."""
    jobs = []
    for bi in _core_blocks(c):
        start = 128 * (bi + 1)
        width = N - start
        for q in range(-(-width // 512)):
            cs = start + 512 * q
            jobs.append((bi, cs, min(512, N - cs)))
    assert len(jobs) == NJOBS
    return jobs


def _build_nc():
    if "nc" in _CACHE:
        return _CACHE["nc"]
    dt = mybir.dt
    nc = bacc.Bacc(
        "TRN2", target_bir_lowering=False, debug=False, num_devices=NCORES
    )
    lhs_d = nc.dram_tensor("lhs", [K, NJOBS * 128], dt.bfloat16, kind="ExternalInput").ap()
    rhs_d = nc.dram_tensor("rhs", [K, NJOBS * 512], dt.bfloat16, kind="ExternalInput").ap()
    lhsd_d = nc.dram_tensor("lhsd", [K, NDIAG * 128], dt.bfloat16, kind="ExternalInput").ap()
    rhsd_d = nc.dram_tensor("rhsd", [K, NDIAG * 128], dt.bfloat16, kind="ExternalInput").ap()
    stats_d = nc.dram_tensor("stats", [128, NSTAT], dt.float32, kind="ExternalOutput").ap()

    with tile.TileContext(nc) as tc:
        with (
            tc.tile_pool(name="const", bufs=1) as cpool,
            tc.tile_pool(name="rchunk", bufs=NGROUPS) as rpool,
            tc.tile_pool(name="psum", bufs=2, space="PSUM") as ppool,
            tc.tile_pool(name="stats", bufs=1) as spool,
        ):
            stats = spool.tile([128, NSTAT], dt.float32)
            lhs = cpool.tile([K, NJOBS * 128], dt.bfloat16)
            nc.sync.dma_start(lhs[:], lhs_d[:])
            lhsd = cpool.tile([K, NDIAG * 128], dt.bfloat16)
            nc.sync.dma_start(lhsd[:], lhsd_d[:])
            rhsd = cpool.tile([K, NDIAG * 128], dt.bfloat16)
            nc.sync.dma_start(rhsd[:], rhsd_d[:])

            chunks = []
            for g in range(NGROUPS):
                w = 2048 if g < NGROUPS - 1 else 1024
                ch = rpool.tile([K, w], dt.bfloat16, tag="rchunk")
                nc.sync.dma_start(ch[:], rhs_d[:, g * 2048 : g * 2048 + w])
                chunks.append((ch, w))

            for g, (ch, w) in enumerate(chunks):
                ps = ppool.tile([128, w], dt.float32, tag="ps")
                for j in range(w // 512):
                    m = 4 * g + j
                    nc.tensor.matmul(
                        ps[:, j * 512 : (j + 1) * 512],
                        lhs[:, m * 128 : (m + 1) * 128],
                        ch[:, j * 512 : (j + 1) * 512],
                        start=True,
                        stop=True,
                    )
                nc.scalar.activation(
                    ps[:],
                    ps[:],
                    mybir.ActivationFunctionType.Tanh,
                    scale=TEMP_INV,
                    accum_out=stats[:, g : g + 1],
                )

            psd = ppool.tile([128, NDIAG * 128], dt.float32, tag="ps")
            for q in range(NDIAG):
                nc.tensor.matmul(
                    psd[:, q * 128 : (q + 1) * 128],
                    lhsd[:, q * 128 : (q + 1) * 128],
                    rhsd[:, q * 128 : (q + 1) * 128],
                    start=True,
                    stop=True,
                )
            nc.scalar.activation(
                psd[:],
                psd[:],
                mybir.ActivationFunctionType.Tanh,
                scale=TEMP_INV,
                accum_out=stats[:, NGROUPS : NGROUPS + 1],
            )

            nc.sync.dma_start(stats_d[:], stats[:])

    nc.compile()
    _CACHE["nc"] = nc
    return nc


def _split_bf16(x):
    hi = x.astype(ml_dtypes.bfloat16).astype(np.float32)
    lo = (x - hi).astype(ml_dtypes.bfloat16).astype(np.float32)
    return hi, lo


def _factor_rows(p, t):
    u = p * t
    ones = np.ones_like(p)
    a_rows, b_rows = [], []
    for a, b in zip((ones, u, p, t), (u, ones, -t, -p)):
        ah, al = _split_bf16(a)
        bh, bl = _split_bf16(b)
        a_rows += [ah, ah, al, al]
        b_rows += [bh, bl, bh, bl]
    A = np.stack(a_rows).astype(ml_dtypes.bfloat16)  # [16, N]
    B = np.stack(b_rows).astype(ml_dtypes.bfloat16)  # [16, N]
    return A, B


def _in_maps(pred, target):
    p = np.asarray(pred, dtype=np.float32).reshape(-1)
    t = np.asarray(target, dtype=np.float32).reshape(-1)
    assert p.size == N and t.size == N
    A, B = _factor_rows(p, t)
    in_maps = []
    for c in range(NCORES):
        lhs = np.zeros((K, NJOBS * 128), ml_dtypes.bfloat16)
        rhs = np.zeros((K, NJOBS * 512), ml_dtypes.bfloat16)
        for m, (bi, cs, w) in enumerate(_jobs_for_core(c)):
            lhs[:, m * 128 : (m + 1) * 128] = A[:, 128 * bi : 128 * (bi + 1)]
            rhs[:, m * 512 : m * 512 + w] = B[:, cs : cs + w]
        lhsd = np.zeros((K, NDIAG * 128), ml_dtypes.bfloat16)
        rhsd = np.zeros((K, NDIAG * 128), ml_dtypes.bfloat16)
        for q, bi in enumerate(_core_blocks(c)):
            lhsd[:, q * 128 : (q + 1) * 128] = A[:, 128 * bi : 128 * (bi + 1)]
            rhsd[:, q * 128 : (q + 1) * 128] = B[:, 128 * bi : 128 * (bi + 1)]
        in_maps.append({"lhs": lhs, "rhs": rhs, "lhsd": lhsd, "rhsd": rhsd})
    return in_maps


def _reduce(stats_list):
    total = 0.0
    for stats in stats_list:
        s = np.asarray(stats, dtype=np.float64)
        total += s[:, :NGROUPS].sum() + 0.5 * s[:, NGROUPS].sum()
    n_pairs = N * (N - 1) / 2.0
    return np.asarray(total / n_pairs, dtype=np.float32)


def run(pred, target, trace=False):
    nc = _build_nc()
    in_maps = _in_maps(pred, target)
    r = run_bass_kernel_spmd(nc, in_maps, list(range(NCORES)), trace=trace)
    tau = _reduce([res["stats"] for res in r.results])
    return tau, r


def kernel(pred, target):
    tau, _ = run(pred, target, trace=False)
    return tau


# revision 2
# speedup vs baseline: 1.0671x; 1.0671x over previous
"""Trainium2 Bass kernel: DifferentiableKendallTau loss.

Reference computes tau = mean over strict-upper-triangle of
tanh((p_j - p_i) * (t_j - t_i) / T) for the flattened n=8192 inputs.

Device strategy (8 NeuronCores, SPMD — one program, per-core data):
  * M[i,j] = (p_j-p_i)(t_j-t_i) is rank-4:  M = 1*u^T + u*1^T - p*t^T - t*p^T
    with u = p*t.  Each fp32 factor is split hi+lo into bf16 (products are
    exact, PSUM accumulates fp32) -> a rank-16 bf16 matmul reproduces M to
    ~1e-7 relative.
  * TensorE builds 128x512 blocks of M into PSUM (K=16 matmuls).
  * ScalarE computes tanh(10*x) in-place over each [128,2048] PSUM window
    with accum_out reduction into a stats column (17 uniform windows).
  * Triangle: each core covers 8 of the 64 row-blocks (balanced pairing
    bi=k / bi=63-k), columns strictly right of the diagonal block
    (zero-padded to 512-col chunks; tanh(0)=0 so padding is free).  The 8
    diagonal blocks ride in the last window at weight 1; VectorE re-reduces
    their tanh values so the host can subtract the 0.5 overcount.
  * Per-group inputs arrive as one packed "slab" DMA (weights + columns),
    so compute starts as soon as slab 0 lands.
  * Host sums the tiny per-core stats and divides by the pair count.
"""

import numpy as np
import ml_dtypes

import concourse.bass as bass
import concourse.bacc as bacc
import concourse.tile as tile
from concourse import mybir
from concourse.bass_utils import run_bass_kernel_spmd

N = 8192
NCORES = 8
TEMP_INV = 10.0          # 1 / TEMPERATURE
K = 16                   # rank after bf16 hi/lo split of 4 fp32 factors
NJOBS = 66               # 512-col off-diag jobs per core (same on all cores)
NGROUPS = 17             # ACT windows of [128, 2048]
NDIAG = 8                # diagonal 128x128 blocks per core
NSTAT = 18               # 17 accum cols + 1 diag-correction col

GSLAB = 2560             # groups 0..15: 512 lhs cols + 2048 rhs cols
G16_OFF = 16 * GSLAB
# group 16 layout (cols within its slab): 2 main lhs | 8 diag lhs | 2 main rhs | 8 diag rhs
G16_LM, G16_LD, G16_RM, G16_RD, G16_SLAB = 0, 256, 1280, 2304, 3328
SLAB_COLS = G16_OFF + G16_SLAB

_CACHE = {}


def _core_blocks(c):
    ks = [4 * c + r for r in range(4)]
    return ks + [63 - k for k in ks]


def _jobs_for_core(c):
    """(row_block, col_start, width<=512) jobs covering columns strictly right
    of each row-block's diagonal block. 66 jobs for every core."""
    jobs = []
    for bi in _core_blocks(c):
        start = 128 * (bi + 1)
        width = N - start
        for q in range(-(-width // 512)):
            cs = start + 512 * q
            jobs.append((bi, cs, min(512, N - cs)))
    assert len(jobs) == NJOBS
    return jobs


def _build_nc():
    if "nc" in _CACHE:
        return _CACHE["nc"]
    dt = mybir.dt
    nc = bacc.Bacc(
        "TRN2", target_bir_lowering=False, debug=False, num_devices=NCORES
    )
    slab_d = nc.dram_tensor("slab", [K, SLAB_COLS], dt.bfloat16, kind="ExternalInput").ap()
    stats_d = nc.dram_tensor("stats", [128, NSTAT], dt.float32, kind="ExternalOutput").ap()

    with tile.TileContext(nc) as tc:
        with (
            tc.tile_pool(name="slabs", bufs=NGROUPS) as lpool,
            tc.tile_pool(name="psum", bufs=2, space="PSUM") as ppool,
            tc.tile_pool(name="stats", bufs=1) as spool,
        ):
            stats = spool.tile([128, NSTAT], dt.float32)

            for g in range(NGROUPS - 1):
                sg = lpool.tile([K, GSLAB], dt.bfloat16, tag="slab")
                nc.sync.dma_start(sg[:], slab_d[:, g * GSLAB : (g + 1) * GSLAB])
                ps = ppool.tile([128, 2048], dt.float32, tag="ps")
                for j in range(4):
                    nc.tensor.matmul(
                        ps[:, j * 512 : (j + 1) * 512],
                        sg[:, j * 128 : (j + 1) * 128],
                        sg[:, 512 + j * 512 : 512 + (j + 1) * 512],
                        start=True,
                        stop=True,
                    )
                nc.scalar.activation(
                    ps[:],
                    ps[:],
                    mybir.ActivationFunctionType.Tanh,
                    scale=TEMP_INV,
                    accum_out=stats[:, g : g + 1],
                )

            # last group: 2 main jobs + 8 diagonal blocks
            g = NGROUPS - 1
            sg = lpool.tile([K, G16_SLAB], dt.bfloat16, tag="slab16")
            nc.sync.dma_start(sg[:], slab_d[:, G16_OFF : G16_OFF + G16_SLAB])
            ps = ppool.tile([128, 2048], dt.float32, tag="ps")
            for j in range(2):
                nc.tensor.matmul(
                    ps[:, j * 512 : (j + 1) * 512],
                    sg[:, G16_LM + j * 128 : G16_LM + (j + 1) * 128],
                    sg[:, G16_RM + j * 512 : G16_RM + (j + 1) * 512],
                    start=True,
                    stop=True,
                )
            for q in range(NDIAG):
                nc.tensor.matmul(
                    ps[:, 1024 + q * 128 : 1024 + (q + 1) * 128],
                    sg[:, G16_LD + q * 128 : G16_LD + (q + 1) * 128],
                    sg[:, G16_RD + q * 128 : G16_RD + (q + 1) * 128],
                    start=True,
                    stop=True,
                )
            nc.scalar.activation(
                ps[:],
                ps[:],
                mybir.ActivationFunctionType.Tanh,
                scale=TEMP_INV,
                accum_out=stats[:, g : g + 1],
            )
            # tanh values are in-place in PSUM: re-reduce the diag half so the
            # host can subtract the 0.5x overcount.
            nc.vector.tensor_reduce(
                stats[:, NSTAT - 1 : NSTAT],
                ps[:, 1024:2048],
                mybir.AxisListType.X,
                mybir.AluOpType.add,
            )

            nc.sync.dma_start(stats_d[:], stats[:])

    nc.compile()
    _CACHE["nc"] = nc
    return nc


def _split_bf16(x):
    hi = x.astype(ml_dtypes.bfloat16).astype(np.float32)
    lo = (x - hi).astype(ml_dtypes.bfloat16).astype(np.float32)
    return hi, lo


def _factor_rows(p, t):
    u = p * t
    ones = np.ones_like(p)
    a_rows, b_rows = [], []
    for a, b in zip((ones, u, p, t), (u, ones, -t, -p)):
        ah, al = _split_bf16(a)
        bh, bl = _split_bf16(b)
        a_rows += [ah, ah, al, al]
        b_rows += [bh, bl, bh, bl]
    A = np.stack(a_rows).astype(ml_dtypes.bfloat16)  # [16, N]
    B = np.stack(b_rows).astype(ml_dtypes.bfloat16)  # [16, N]
    return A, B


def _in_maps(pred, target):
    p = np.asarray(pred, dtype=np.float32).reshape(-1)
    t = np.asarray(target, dtype=np.float32).reshape(-1)
    assert p.size == N and t.size == N
    A, B = _factor_rows(p, t)
    in_maps = []
    for c in range(NCORES):
        jobs = _jobs_for_core(c)
        slab = np.zeros((K, SLAB_COLS), ml_dtypes.bfloat16)
        for g in range(NGROUPS - 1):
            base = g * GSLAB
            for j in range(4):
                bi, cs, w = jobs[4 * g + j]
                slab[:, base + j * 128 : base + (j + 1) * 128] = A[:, 128 * bi : 128 * (bi + 1)]
                slab[:, base + 512 + j * 512 : base + 512 + j * 512 + w] = B[:, cs : cs + w]
        for j in range(2):
            bi, cs, w = jobs[64 + j]
            slab[:, G16_OFF + G16_LM + j * 128 : G16_OFF + G16_LM + (j + 1) * 128] = (
                A[:, 128 * bi : 128 * (bi + 1)]
            )
            slab[:, G16_OFF + G16_RM + j * 512 : G16_OFF + G16_RM + j * 512 + w] = (
                B[:, cs : cs + w]
            )
        for q, bi in enumerate(_core_blocks(c)):
            slab[:, G16_OFF + G16_LD + q * 128 : G16_OFF + G16_LD + (q + 1) * 128] = (
                A[:, 128 * bi : 128 * (bi + 1)]
            )
            slab[:, G16_OFF + G16_RD + q * 128 : G16_OFF + G16_RD + (q + 1) * 128] = (
                B[:, 128 * bi : 128 * (bi + 1)]
            )
        in_maps.append({"slab": slab})
    return in_maps


def _reduce(stats_list):
    total = 0.0
    for stats in stats_list:
        s = np.asarray(stats, dtype=np.float64)
        total += s[:, : NGROUPS].sum() - 0.5 * s[:, NSTAT - 1].sum()
    n_pairs = N * (N - 1) / 2.0
    return np.asarray(total / n_pairs, dtype=np.float32)


def run(pred, target, trace=False):
    nc = _build_nc()
    in_maps = _in_maps(pred, target)
    r = run_bass_kernel_spmd(nc, in_maps, list(range(NCORES)), trace=trace)
    tau = _reduce([res["stats"] for res in r.results])
    return tau, r


def kernel(pred, target):
    tau, _ = run(pred, target, trace=False)
    return tau
